# revision 30
# baseline (speedup 1.0000x reference)
"""Trainium2 Bass kernel for fused QKV-projection + multi-head attention.

Problem: x[2,2048,1024] @ W_qkv[1024,3072] + b -> split q/k/v -> 16 heads of
dim 64 -> softmax(q k^T / 8) v -> [2,2048,1024].

Sharding (8 cores): data-parallel over batch (2) x tensor-parallel over head
groups (4 heads per core).  Each core computes a disjoint output slice
[2048, 256]; no collectives are needed.

Design notes:
- Matmul operands are bf16 (fp32 PSUM accumulation): full-rate PE with
  overlapped weight loads. x is pre-transposed and pre-cast on the host, so
  no on-device transposes are needed for the projection.
- qT/kT live in [head-dim, t] layout packed as head PAIRS on the partition
  axis; the scores matmuls use 64-row array tiling (tile_position (0,0) /
  (64,0)) so both heads of a pair run concurrently on the PE array.
- scoresT [k, q] layout: softmax denominator = ones-column appended to V
  ([E^T V | E^T 1] in one PSUM accumulation).  exp has no max-subtraction:
  scores are bounded (~[-3.3, 3.3]) for this problem's scale.
- The final division by the denominator happens on the host; the kernel
  returns the numerator and denominators.
"""

import sys

sys.path.insert(0, "/opt/trn_rl_repo")

import numpy as np

import concourse.bacc as bacc
import concourse.bass as bass
import concourse.mybir as mybir
import concourse.tile as tile
from concourse.bass import ts
from concourse.masks import make_identity

P = 128
T = 2048
D = 1024
NH = 4          # heads per core
HD = 64         # head dim
TB = T // P     # 16 t-blocks
CB = D // P     # 8 c-blocks
QKV_COLS = 3 * NH * HD  # 768 per core
F32 = mybir.dt.float32
BF16 = mybir.dt.bfloat16

_CACHED = {}


def build_bass(finalize=True):
    nc = bacc.Bacc()

    xT_d = nc.dram_tensor("xT", [D, T], BF16, kind="ExternalInput")
    w_d = nc.dram_tensor("w", [D, QKV_COLS], BF16, kind="ExternalInput")
    bqk_d = nc.dram_tensor("bqk", [P, 4], F32, kind="ExternalInput")
    bv_d = nc.dram_tensor("bv", [1, NH * HD], F32, kind="ExternalInput")
    y_d = nc.dram_tensor("y", [T, NH * HD], F32, kind="ExternalOutput")
    den_d = nc.dram_tensor("den", [NH, T], F32, kind="ExternalOutput")

    with tile.TileContext(nc) as tc:
        with (
            tc.tile_pool(name="persist", bufs=1) as persist,
            tc.tile_pool(name="small", bufs=2) as small,
            tc.tile_pool(name="ystage", bufs=4) as ystage,
            tc.tile_pool(name="epool", bufs=3) as epool,
            tc.tile_pool(name="ps_s", bufs=1, space="PSUM") as ps_s,
            tc.tile_pool(name="ps_y", bufs=1, space="PSUM") as ps_y,
        ):
            ident = persist.tile([P, P], F32)
            make_identity(nc, ident)

            # [p, pair, t]; head 2*pr at partitions 0:64, 2*pr+1 at 64:128
            qT = [persist.tile([P, T], BF16, name=f"qT{i}") for i in range(2)]
            kT = [persist.tile([P, T], BF16, name=f"kT{i}") for i in range(2)]
            # V' with ones column per head: [t-part, tb, h, 65]
            vv = persist.tile([P, TB, NH, HD + 1], BF16)
            bqk_sb = persist.tile([P, 4], F32)
            bvb = persist.tile([P, NH * HD], F32)
            # unnormalized numerator, transposed layout [d-part, t], per pair
            yT = [persist.tile([P, T], F32, name=f"yT{i}") for i in range(2)]

            nc.vector.memset(vv[:, :, :, HD : HD + 1], 1.0)
            nc.sync.dma_start(out=bqk_sb[:], in_=bqk_d[:, :])
            nc.gpsimd.dma_start(
                out=bvb[:], in_=bv_d[0:1, :].to_broadcast((P, NH * HD))
            )

            w_sb = persist.tile([P, CB, QKV_COLS], BF16)
            nc.sync.dma_start(
                out=w_sb[:], in_=w_d[:, :].rearrange("(cb p) col -> p cb col", p=P)
            )
            xT_sb = persist.tile([P, CB, T], BF16)
            nc.sync.dma_start(
                out=xT_sb[:], in_=xT_d[:, :].rearrange("(cb p) t -> p cb t", p=P)
            )

            # ---------------- QKV projection --------------------------------
            # v first so the AV matmuls unblock early
            for tb in range(TB):
                pv = ps_y.tile([P, NH * HD], F32, tag=["Y0", "Y1"][tb % 2])
                for cb in range(CB):
                    nc.tensor.matmul(
                        pv[:],
                        lhsT=xT_sb[:, cb, ts(tb, P)],
                        rhs=w_sb[:, cb, 2 * NH * HD : 3 * NH * HD],
                        start=(cb == 0),
                        stop=(cb == CB - 1),
                    )
                nc.vector.tensor_tensor(
                    out=vv[:, tb, :, 0:HD],
                    in0=pv[:].rearrange("p (a b) -> p a b", a=NH),
                    in1=bvb[:].rearrange("p (a b) -> p a b", a=NH),
                    op=mybir.AluOpType.add,
                )

            # q/k projection -> qT/kT (transposed layout, bf16)
            for ct in range(4):  # 0,1: q pairs; 2,3: k pairs
                dst = qT[ct] if ct < 2 else kT[ct - 2]
                for tc2 in range(4):  # 512-wide t-chunks
                    if tc2 % 2 == 0:
                        pqk = ps_s.tile([P, 512], F32, tag="S")
                    else:
                        pqk = ps_y.tile([P, 512], F32, tag="Y1")
                    for cb in range(CB):
                        nc.tensor.matmul(
                            pqk[:],
                            lhsT=w_sb[:, cb, ts(ct, P)],
                            rhs=xT_sb[:, cb, ts(tc2, 512)],
                            start=(cb == 0),
                            stop=(cb == CB - 1),
                        )
                    nc.vector.tensor_scalar_add(
                        out=dst[:, ts(tc2, 512)],
                        in0=pqk[:],
                        scalar1=bqk_sb[:, ct : ct + 1],
                    )

            # ---------------- attention -------------------------------------
            for pr in range(2):
                for qh in range(2):  # 1024-wide q halves
                    pY = [
                        ps_y.tile([HD + 1, 1024], F32, tag=f"Y{s}", name=f"pY{s}")
                        for s in range(2)
                    ]
                    for kb in range(TB):
                        # pair's scoresT in ONE 4-bank psum tile: head A in
                        # cols 0:1024, head B in 1024:2048 -> single big exp
                        pS = ps_s.tile([P, 2048], F32, tag="S")
                        for i in range(2):
                            for s in range(2):
                                nc.tensor.matmul(
                                    pS[:, s * 1024 + i * 512 : s * 1024 + (i + 1) * 512],
                                    lhsT=kT[pr][s * 64 : (s + 1) * 64, ts(kb, P)],
                                    rhs=qT[pr][
                                        s * 64 : (s + 1) * 64,
                                        qh * 1024 + i * 512 : qh * 1024 + (i + 1) * 512,
                                    ],
                                    start=True,
                                    stop=True,
                                    tile_position=(s * 64, 0),
                                )
                        eT = epool.tile([P, 2048], BF16, tag="E")
                        nc.scalar.activation(
                            out=eT[:],
                            in_=pS[:],
                            func=mybir.ActivationFunctionType.Exp,
                            scale=0.125,
                        )
                        for s in range(2):
                            for i in range(2):
                                nc.tensor.matmul(
                                    pY[s][:, ts(i, 512)],
                                    lhsT=vv[:, kb, 2 * pr + s, :],
                                    rhs=eT[:, s * 1024 + i * 512 : s * 1024 + (i + 1) * 512],
                                    start=(kb == 0),
                                    stop=(kb == TB - 1),
                                )
                    for s in range(2):
                        nc.vector.tensor_copy(
                            out=yT[pr][s * 64 : (s + 1) * 64, ts(qh, 1024)],
                            in_=pY[s][0:HD, :],
                        )
                        dsb = small.tile([1, 1024], F32)
                        nc.vector.tensor_copy(out=dsb[:], in_=pY[s][HD : HD + 1, :])
                        nc.sync.dma_start(
                            out=den_d[2 * pr + s : 2 * pr + s + 1, ts(qh, 1024)],
                            in_=dsb[:],
                        )

                # transpose this pair back to [t, d] and store (overlaps the
                # next pair's attention; borrows the Y0 psum slot so the S
                # slot stays free for the next pair's scores)
                for tb in range(TB):
                    pT = ps_y.tile([P, 1024], F32, tag="Y0")
                    nc.tensor.transpose(pT[:, 0:P], yT[pr][:, ts(tb, P)], ident)
                    yst = ystage.tile([P, P], F32)
                    nc.vector.tensor_copy(out=yst[:], in_=pT[:, 0:P])
                    nc.sync.dma_start(out=y_d[ts(tb, P), ts(pr, P)], in_=yst[:])

    if finalize:
        nc.finalize()
    return nc


def _shard_inputs(x, W_qkv, b_qkv):
    """Build per-core input maps. Core c: batch c//4, head group c%4."""
    import ml_dtypes

    x = np.asarray(x, dtype=np.float32)
    W = np.asarray(W_qkv, dtype=np.float32)
    b = np.asarray(b_qkv, dtype=np.float32)
    bf = ml_dtypes.bfloat16
    xT = [np.ascontiguousarray(x[bi].T.astype(bf)) for bi in range(2)]
    in_maps = []
    for c in range(8):
        bi, hg = c // 4, c % 4
        cs = hg * 256  # column start within each of q/k/v blocks
        w_core = np.concatenate(
            [
                W[:, cs : cs + 256],
                W[:, D + cs : D + cs + 256],
                W[:, 2 * D + cs : 2 * D + cs + 256],
            ],
            axis=1,
        ).astype(bf)
        bqk = np.concatenate([b[cs : cs + 256], b[D + cs : D + cs + 256]])
        bqk = np.ascontiguousarray(bqk.reshape(4, 128).T)
        bv = np.ascontiguousarray(b[2 * D + cs : 2 * D + cs + 256].reshape(1, 256))
        in_maps.append(
            {
                "xT": xT[bi],
                "w": np.ascontiguousarray(w_core),
                "bqk": bqk,
                "bv": bv,
            }
        )
    return in_maps


def kernel(x, W_qkv, b_qkv, trace=False):
    from concourse.bass_utils import run_bass_kernel_spmd

    if "nc" not in _CACHED:
        _CACHED["nc"] = build_bass()
    nc = _CACHED["nc"]

    in_maps = _shard_inputs(x, W_qkv, b_qkv)
    res = run_bass_kernel_spmd(nc, in_maps, list(range(8)), trace=trace)
    _CACHED["last_result"] = res

    out = np.empty((2, T, D), dtype=np.float32)
    for c in range(8):
        bi, hg = c // 4, c % 4
        y_raw = res.results[c]["y"]  # [T, 256] unnormalized
        den = res.results[c]["den"]  # [4, T]
        y = y_raw.reshape(T, NH, HD) / den.T[:, :, None]
        out[bi, :, hg * 256 : (hg + 1) * 256] = y.reshape(T, NH * HD)
    return out


if __name__ == "__main__":
    nc = build_bass()
    print("built ok")


# revision 34
# speedup vs baseline: 1.4033x; 1.4033x over previous
"""Trainium2 Bass kernel for fused QKV-projection + multi-head attention.

Problem: x[2,2048,1024] @ W_qkv[1024,3072] + b -> split q/k/v -> 16 heads of
dim 64 -> softmax(q k^T / 8) v -> [2,2048,1024].

Sharding (8 cores): data-parallel over batch (2) x tensor-parallel over head
groups (4 heads per core).  Each core computes a disjoint output slice
[2048, 256]; no collectives are needed.

Design notes:
- Matmul operands are bf16 (fp32 PSUM accumulation): full-rate PE with
  overlapped weight loads. x is pre-transposed and pre-cast on the host, so
  no on-device transposes are needed for the projection.
- qT/kT live in [head-dim, t] layout packed as head PAIRS on the partition
  axis; the scores matmuls use 64-row array tiling (tile_position (0,0) /
  (64,0)) so both heads of a pair run concurrently on the PE array.
- scoresT [k, q] layout: softmax denominator = ones-column appended to V
  ([E^T V | E^T 1] in one PSUM accumulation).  exp has no max-subtraction:
  scores are bounded (~[-3.3, 3.3]) for this problem's scale.
- The final division by the denominator happens on the host; the kernel
  returns the numerator and denominators.
"""

import sys

sys.path.insert(0, "/opt/trn_rl_repo")

import numpy as np

import concourse.bacc as bacc
import concourse.bass as bass
import concourse.mybir as mybir
import concourse.tile as tile
from concourse.bass import ts
from concourse.masks import make_identity

P = 128
T = 2048
D = 1024
NH = 4          # heads per core
HD = 64         # head dim
TB = T // P     # 16 t-blocks
CB = D // P     # 8 c-blocks
QKV_COLS = 3 * NH * HD  # 768 per core
F32 = mybir.dt.float32
BF16 = mybir.dt.bfloat16

_CACHED = {}


def build_bass(finalize=True):
    nc = bacc.Bacc()

    xT_d = nc.dram_tensor("xT", [D, T], BF16, kind="ExternalInput")
    w_d = nc.dram_tensor("w", [D, QKV_COLS], BF16, kind="ExternalInput")
    bqk_d = nc.dram_tensor("bqk", [P, 4], F32, kind="ExternalInput")
    bv_d = nc.dram_tensor("bv", [1, NH * HD], F32, kind="ExternalInput")
    y_d = nc.dram_tensor("y", [T, NH * HD], F32, kind="ExternalOutput")
    den_d = nc.dram_tensor("den", [NH, T], F32, kind="ExternalOutput")

    with tile.TileContext(nc) as tc:
        with (
            tc.tile_pool(name="persist", bufs=1) as persist,
            tc.tile_pool(name="small", bufs=2) as small,
            tc.tile_pool(name="ystage", bufs=4) as ystage,
            tc.tile_pool(name="epool", bufs=3) as epool,
            tc.tile_pool(name="ps_s", bufs=1, space="PSUM") as ps_s,
            tc.tile_pool(name="ps_y", bufs=1, space="PSUM") as ps_y,
        ):
            ident = persist.tile([P, P], F32)
            make_identity(nc, ident)

            # kT: [p, t] per pair; head 2*pr at partitions 0:64, 2*pr+1 at 64:128
            kT = [persist.tile([P, T], BF16, name=f"kT{i}") for i in range(2)]
            # qT: [p, h, t] zero-padded per head: head h's 64 dims live at
            # partitions (h%2)*64..+64, the other 64 partitions stay zero so a
            # full-128 matmul against the kT pair tile selects only head h
            qT = persist.tile([P, NH, T], BF16)
            nc.vector.memset(qT[:], 0.0)
            # V' with ones column per head: [t-part, tb, h, 65]
            vv = persist.tile([P, TB, NH, HD + 1], BF16)
            bqk_sb = persist.tile([P, 4], F32)
            bvb = persist.tile([P, NH * HD], F32)
            # unnormalized numerator, transposed layout [d-part, t], per pair
            yT = [persist.tile([P, T], F32, name=f"yT{i}") for i in range(2)]

            nc.vector.memset(vv[:, :, :, HD : HD + 1], 1.0)
            nc.sync.dma_start(out=bqk_sb[:], in_=bqk_d[:, :])
            nc.gpsimd.dma_start(
                out=bvb[:], in_=bv_d[0:1, :].to_broadcast((P, NH * HD))
            )

            w_sb = persist.tile([P, CB, QKV_COLS], BF16)
            nc.sync.dma_start(
                out=w_sb[:], in_=w_d[:, :].rearrange("(cb p) col -> p cb col", p=P)
            )
            xT_sb = persist.tile([P, CB, T], BF16)
            nc.sync.dma_start(
                out=xT_sb[:], in_=xT_d[:, :].rearrange("(cb p) t -> p cb t", p=P)
            )

            # ---------------- QKV projection --------------------------------
            # v first so the AV matmuls unblock early
            for tb in range(TB):
                pv = ps_y.tile([P, NH * HD], F32, tag=["Y0", "Y1"][tb % 2])
                for cb in range(CB):
                    nc.tensor.matmul(
                        pv[:],
                        lhsT=xT_sb[:, cb, ts(tb, P)],
                        rhs=w_sb[:, cb, 2 * NH * HD : 3 * NH * HD],
                        start=(cb == 0),
                        stop=(cb == CB - 1),
                    )
                nc.vector.tensor_tensor(
                    out=vv[:, tb, :, 0:HD],
                    in0=pv[:].rearrange("p (a b) -> p a b", a=NH),
                    in1=bvb[:].rearrange("p (a b) -> p a b", a=NH),
                    op=mybir.AluOpType.add,
                )

            # q/k projection -> qT/kT (transposed layout, bf16)
            for ct in range(4):  # 0,1: q pairs; 2,3: k pairs
                for tc2 in range(4):  # 512-wide t-chunks
                    pqk = ps_s.tile([P, 512], F32, tag=f"S{tc2 % 2}", name="pqk")
                    for cb in range(CB):
                        nc.tensor.matmul(
                            pqk[:],
                            lhsT=w_sb[:, cb, ts(ct, P)],
                            rhs=xT_sb[:, cb, ts(tc2, 512)],
                            start=(cb == 0),
                            stop=(cb == CB - 1),
                        )
                    if ct < 2:
                        for s in range(2):
                            nc.vector.tensor_scalar_add(
                                out=qT[
                                    s * 64 : (s + 1) * 64, 2 * ct + s, ts(tc2, 512)
                                ],
                                in0=pqk[s * 64 : (s + 1) * 64, :],
                                scalar1=bqk_sb[s * 64 : (s + 1) * 64, ct : ct + 1],
                            )
                    else:
                        nc.vector.tensor_scalar_add(
                            out=kT[ct - 2][:, ts(tc2, 512)],
                            in0=pqk[:],
                            scalar1=bqk_sb[:, ct : ct + 1],
                        )

            # ---------------- attention -------------------------------------
            # Software pipeline per kb: scores(kb) -> exp(kb) on ACT while the
            # PE runs AV(kb-1).  AV is issued AFTER the next scores so the
            # in-order PE queue never stalls behind the exp it feeds.
            for pr in range(2):
                for qh in range(2):  # 1024-wide q halves
                    pY = [
                        ps_y.tile([HD + 1, 1024], F32, tag=f"Y{s}", name=f"pY{s}")
                        for s in range(2)
                    ]

                    def issue_av(kb, eprev):
                        for s in range(2):
                            for i in range(2):
                                nc.tensor.matmul(
                                    pY[s][:, ts(i, 512)],
                                    lhsT=vv[:, kb, 2 * pr + s, :],
                                    rhs=eprev[s][:, ts(i, 512)],
                                    start=(kb == 0),
                                    stop=(kb == TB - 1),
                                )

                    prev = None
                    for kb in range(TB):
                        pS = [
                            ps_s.tile([P, 1024], F32, tag=f"S{s}", name=f"pS{s}")
                            for s in range(2)
                        ]
                        for s in range(2):
                            for i in range(2):
                                nc.tensor.matmul(
                                    pS[s][:, ts(i, 512)],
                                    lhsT=kT[pr][:, ts(kb, P)],
                                    rhs=qT[
                                        :,
                                        2 * pr + s,
                                        qh * 1024 + i * 512 : qh * 1024 + (i + 1) * 512,
                                    ],
                                    start=True,
                                    stop=True,
                                )
                        eT = [
                            epool.tile([P, 1024], BF16, tag=f"E{s}", name=f"eT{s}")
                            for s in range(2)
                        ]
                        for s in range(2):
                            nc.scalar.activation(
                                out=eT[s][:],
                                in_=pS[s][:],
                                func=mybir.ActivationFunctionType.Exp,
                                scale=0.125,
                            )
                        if prev is not None:
                            issue_av(kb - 1, prev)
                        prev = eT
                    issue_av(TB - 1, prev)
                    for s in range(2):
                        nc.vector.tensor_copy(
                            out=yT[pr][s * 64 : (s + 1) * 64, ts(qh, 1024)],
                            in_=pY[s][0:HD, :],
                        )
                        dsb = small.tile([1, 1024], F32)
                        nc.vector.tensor_copy(out=dsb[:], in_=pY[s][HD : HD + 1, :])
                        nc.sync.dma_start(
                            out=den_d[2 * pr + s : 2 * pr + s + 1, ts(qh, 1024)],
                            in_=dsb[:],
                        )

                # transpose this pair back to [t, d] and store (overlaps the
                # next pair's attention; borrows the Y0 psum slot so the S
                # slot stays free for the next pair's scores)
                for tb in range(TB):
                    pT = ps_y.tile([P, 1024], F32, tag="Y0")
                    nc.tensor.transpose(pT[:, 0:P], yT[pr][:, ts(tb, P)], ident)
                    yst = ystage.tile([P, P], F32)
                    nc.vector.tensor_copy(out=yst[:], in_=pT[:, 0:P])
                    nc.sync.dma_start(out=y_d[ts(tb, P), ts(pr, P)], in_=yst[:])

    if finalize:
        nc.finalize()
    return nc


def _shard_inputs(x, W_qkv, b_qkv):
    """Build per-core input maps. Core c: batch c//4, head group c%4."""
    import ml_dtypes

    x = np.asarray(x, dtype=np.float32)
    W = np.asarray(W_qkv, dtype=np.float32)
    b = np.asarray(b_qkv, dtype=np.float32)
    bf = ml_dtypes.bfloat16
    xT = [np.ascontiguousarray(x[bi].T.astype(bf)) for bi in range(2)]
    in_maps = []
    for c in range(8):
        bi, hg = c // 4, c % 4
        cs = hg * 256  # column start within each of q/k/v blocks
        w_core = np.concatenate(
            [
                W[:, cs : cs + 256],
                W[:, D + cs : D + cs + 256],
                W[:, 2 * D + cs : 2 * D + cs + 256],
            ],
            axis=1,
        ).astype(bf)
        bqk = np.concatenate([b[cs : cs + 256], b[D + cs : D + cs + 256]])
        bqk = np.ascontiguousarray(bqk.reshape(4, 128).T)
        bv = np.ascontiguousarray(b[2 * D + cs : 2 * D + cs + 256].reshape(1, 256))
        in_maps.append(
            {
                "xT": xT[bi],
                "w": np.ascontiguousarray(w_core),
                "bqk": bqk,
                "bv": bv,
            }
        )
    return in_maps


def kernel(x, W_qkv, b_qkv, trace=False):
    from concourse.bass_utils import run_bass_kernel_spmd

    if "nc" not in _CACHED:
        _CACHED["nc"] = build_bass()
    nc = _CACHED["nc"]

    in_maps = _shard_inputs(x, W_qkv, b_qkv)
    res = run_bass_kernel_spmd(nc, in_maps, list(range(8)), trace=trace)
    _CACHED["last_result"] = res

    out = np.empty((2, T, D), dtype=np.float32)
    for c in range(8):
        bi, hg = c // 4, c % 4
        y_raw = res.results[c]["y"]  # [T, 256] unnormalized
        den = res.results[c]["den"]  # [4, T]
        y = y_raw.reshape(T, NH, HD) / den.T[:, :, None]
        out[bi, :, hg * 256 : (hg + 1) * 256] = y.reshape(T, NH * HD)
    return out


if __name__ == "__main__":
    nc = build_bass()
    print("built ok")


# revision 38
# speedup vs baseline: 1.7802x; 1.2686x over previous
"""Trainium2 Bass kernel for fused QKV-projection + multi-head attention.

Problem: x[2,2048,1024] @ W_qkv[1024,3072] + b -> split q/k/v -> 16 heads of
dim 64 -> softmax(q k^T / 8) v -> [2,2048,1024].

Sharding (8 cores): data-parallel over batch (2) x tensor-parallel over head
groups (4 heads per core).  Each core computes a disjoint output slice
[2048, 256]; no collectives are needed.

Design notes:
- Matmul operands are bf16 (fp32 PSUM accumulation): full-rate PE with
  overlapped weight loads. x is pre-transposed and pre-cast on the host, so
  no on-device transposes are needed for the projection.
- qT/kT live in [head-dim, t] layout packed as head PAIRS on the partition
  axis; the scores matmuls use 64-row array tiling (tile_position (0,0) /
  (64,0)) so both heads of a pair run concurrently on the PE array.
- scoresT [k, q] layout: softmax denominator = ones-column appended to V
  ([E^T V | E^T 1] in one PSUM accumulation).  exp has no max-subtraction:
  scores are bounded (~[-3.3, 3.3]) for this problem's scale.
- The final division by the denominator happens on the host; the kernel
  returns the numerator and denominators.
"""

import sys

sys.path.insert(0, "/opt/trn_rl_repo")

import numpy as np

import concourse.bacc as bacc
import concourse.bass as bass
import concourse.mybir as mybir
import concourse.tile as tile
from concourse.bass import ts
from concourse.masks import make_identity

P = 128
T = 2048
D = 1024
NH = 4          # heads per core
HD = 64         # head dim
TB = T // P     # 16 t-blocks
CB = D // P     # 8 c-blocks
QKV_COLS = 3 * NH * HD  # 768 per core
F32 = mybir.dt.float32
BF16 = mybir.dt.bfloat16

_CACHED = {}


def build_bass(finalize=True):
    nc = bacc.Bacc()

    xT_d = nc.dram_tensor("xT", [D, T], BF16, kind="ExternalInput")
    w_d = nc.dram_tensor("w", [D, QKV_COLS], BF16, kind="ExternalInput")
    bqk_d = nc.dram_tensor("bqk", [P, 4], F32, kind="ExternalInput")
    bv_d = nc.dram_tensor("bv", [1, NH * HD], F32, kind="ExternalInput")
    y_d = nc.dram_tensor("y", [T, NH * HD], F32, kind="ExternalOutput")
    den_d = nc.dram_tensor("den", [NH, T], F32, kind="ExternalOutput")

    with tile.TileContext(nc) as tc:
        with (
            tc.tile_pool(name="persist", bufs=1) as persist,
            tc.tile_pool(name="small", bufs=2) as small,
            tc.tile_pool(name="ystage", bufs=4) as ystage,
            tc.tile_pool(name="epool", bufs=3) as epool,
            tc.tile_pool(name="ps_s", bufs=1, space="PSUM") as ps_s,
            tc.tile_pool(name="ps_y", bufs=1, space="PSUM") as ps_y,
        ):
            ident = persist.tile([P, P], F32)
            make_identity(nc, ident)

            # kT: [p, t] per pair; head 2*pr at partitions 0:64, 2*pr+1 at 64:128
            kT = [persist.tile([P, T], BF16, name=f"kT{i}") for i in range(2)]
            # qT: [p, t] per head, zero-padded: head h's 64 dims live at
            # partitions (h%2)*64..+64, the other 64 partitions stay zero so a
            # full-128 matmul against the kT pair tile selects only head h
            qT = [persist.tile([P, T], BF16, name=f"qT{h}") for h in range(NH)]
            for h in range(NH):
                nc.vector.memset(qT[h][:], 0.0)
            # V' with ones column per head: [t-part, h, 65], one tile per tb
            vv = [
                persist.tile([P, NH, HD + 1], BF16, name=f"vv{tb}")
                for tb in range(TB)
            ]
            for tb in range(TB):
                nc.vector.memset(vv[tb][:, :, HD : HD + 1], 1.0)
            bqk_sb = persist.tile([P, 4], F32)
            bvb = persist.tile([P, NH * HD], F32)
            # unnormalized numerator, transposed layout [d-part, t], per pair
            yT = [persist.tile([P, T], F32, name=f"yT{i}") for i in range(2)]

            nc.sync.dma_start(out=bqk_sb[:], in_=bqk_d[:, :])
            nc.gpsimd.dma_start(
                out=bvb[:], in_=bv_d[0:1, :].to_broadcast((P, NH * HD))
            )

            w_sb = persist.tile([P, CB, QKV_COLS], BF16)
            nc.sync.dma_start(
                out=w_sb[:], in_=w_d[:, :].rearrange("(cb p) col -> p cb col", p=P)
            )
            # one tile + DMA per c-block so matmuls start on the first chunk
            xTs = [persist.tile([P, T], BF16, name=f"xTs{cb}") for cb in range(CB)]
            for cb in range(CB):
                nc.sync.dma_start(out=xTs[cb][:], in_=xT_d[ts(cb, P), :])

            # ---------------- QKV projection --------------------------------
            def qk_proj(ct):
                for tc2 in range(4):  # 512-wide t-chunks
                    pqk = ps_s.tile([P, 512], F32, tag=f"S{tc2 % 2}", name="pqk")
                    for cb in range(CB):
                        nc.tensor.matmul(
                            pqk[:],
                            lhsT=w_sb[:, cb, ts(ct, P)],
                            rhs=xTs[cb][:, ts(tc2, 512)],
                            start=(cb == 0),
                            stop=(cb == CB - 1),
                        )
                    if ct < 2:
                        for s in range(2):
                            nc.vector.tensor_scalar_add(
                                out=qT[2 * ct + s][
                                    s * 64 : (s + 1) * 64, ts(tc2, 512)
                                ],
                                in0=pqk[s * 64 : (s + 1) * 64, :],
                                scalar1=bqk_sb[s * 64 : (s + 1) * 64, ct : ct + 1],
                            )
                    else:
                        nc.vector.tensor_scalar_add(
                            out=kT[ct - 2][:, ts(tc2, 512)],
                            in0=pqk[:],
                            scalar1=bqk_sb[:, ct : ct + 1],
                        )

            # pair 0's q/k first so its attention can begin ASAP
            qk_proj(0)
            qk_proj(2)
            for tb in range(TB):
                pv = ps_y.tile(
                    [P, NH * HD], F32, tag=["Y0", "Y1"][tb % 2], name="pv"
                )
                for cb in range(CB):
                    nc.tensor.matmul(
                        pv[:],
                        lhsT=xTs[cb][:, ts(tb, P)],
                        rhs=w_sb[:, cb, 2 * NH * HD : 3 * NH * HD],
                        start=(cb == 0),
                        stop=(cb == CB - 1),
                    )
                nc.vector.tensor_tensor(
                    out=vv[tb][:, :, 0:HD],
                    in0=pv[:].rearrange("p (a b) -> p a b", a=NH),
                    in1=bvb[:].rearrange("p (a b) -> p a b", a=NH),
                    op=mybir.AluOpType.add,
                )
            qk_proj(1)
            qk_proj(3)

            # ---------------- attention -------------------------------------
            # Software pipeline per kb: scores(kb) -> exp(kb) on ACT while the
            # PE runs AV(kb-1).  AV is issued AFTER the next scores so the
            # in-order PE queue never stalls behind the exp it feeds.
            for pr in range(2):
                for qh in range(2):  # 1024-wide q halves
                    pY = [
                        ps_y.tile([HD + 1, 1024], F32, tag=f"Y{s}", name=f"pY{s}")
                        for s in range(2)
                    ]

                    def issue_av(kb, eprev):
                        for s in range(2):
                            for i in range(2):
                                nc.tensor.matmul(
                                    pY[s][:, ts(i, 512)],
                                    lhsT=vv[kb][:, 2 * pr + s, :],
                                    rhs=eprev[s][:, ts(i, 512)],
                                    start=(kb == 0),
                                    stop=(kb == TB - 1),
                                )

                    prev = None
                    for kb in range(TB):
                        pS = [
                            ps_s.tile([P, 1024], F32, tag=f"S{s}", name=f"pS{s}")
                            for s in range(2)
                        ]
                        for s in range(2):
                            for i in range(2):
                                nc.tensor.matmul(
                                    pS[s][:, ts(i, 512)],
                                    lhsT=kT[pr][:, ts(kb, P)],
                                    rhs=qT[2 * pr + s][
                                        :,
                                        qh * 1024 + i * 512 : qh * 1024 + (i + 1) * 512,
                                    ],
                                    start=True,
                                    stop=True,
                                )
                        eT = [
                            epool.tile([P, 1024], BF16, tag=f"E{s}", name=f"eT{s}")
                            for s in range(2)
                        ]
                        for s in range(2):
                            nc.scalar.activation(
                                out=eT[s][:],
                                in_=pS[s][:],
                                func=mybir.ActivationFunctionType.Exp,
                                scale=0.125,
                            )
                        if prev is not None:
                            issue_av(kb - 1, prev)
                        prev = eT
                    issue_av(TB - 1, prev)
                    for s in range(2):
                        nc.vector.tensor_copy(
                            out=yT[pr][s * 64 : (s + 1) * 64, ts(qh, 1024)],
                            in_=pY[s][0:HD, :],
                        )
                        dsb = small.tile([1, 1024], F32)
                        nc.vector.tensor_copy(out=dsb[:], in_=pY[s][HD : HD + 1, :])
                        nc.sync.dma_start(
                            out=den_d[2 * pr + s : 2 * pr + s + 1, ts(qh, 1024)],
                            in_=dsb[:],
                        )

                # transpose this pair back to [t, d] and store (overlaps the
                # next pair's attention; borrows the Y psum slots so the S
                # slots stay free for the next pair's scores)
                for g in range(4):
                    pT = ps_y.tile(
                        [P, 512], F32, tag=["Y0", "Y1"][g % 2], name="pT"
                    )
                    for j in range(4):
                        nc.tensor.transpose(
                            pT[:, ts(j, P)], yT[pr][:, ts(4 * g + j, P)], ident
                        )
                    yst = ystage.tile([P, 4, P], F32, name="yst")
                    nc.vector.tensor_copy(out=yst[:], in_=pT[:].rearrange("p (a b) -> p a b", a=4))
                    for j in range(4):
                        nc.sync.dma_start(
                            out=y_d[ts(4 * g + j, P), ts(pr, P)], in_=yst[:, j, :]
                        )

    if finalize:
        nc.finalize()
    return nc


def _shard_inputs(x, W_qkv, b_qkv):
    """Build per-core input maps. Core c: batch c//4, head group c%4."""
    import ml_dtypes

    x = np.asarray(x, dtype=np.float32)
    W = np.asarray(W_qkv, dtype=np.float32)
    b = np.asarray(b_qkv, dtype=np.float32)
    bf = ml_dtypes.bfloat16
    xT = [np.ascontiguousarray(x[bi].T.astype(bf)) for bi in range(2)]
    in_maps = []
    for c in range(8):
        bi, hg = c // 4, c % 4
        cs = hg * 256  # column start within each of q/k/v blocks
        w_core = np.concatenate(
            [
                W[:, cs : cs + 256],
                W[:, D + cs : D + cs + 256],
                W[:, 2 * D + cs : 2 * D + cs + 256],
            ],
            axis=1,
        ).astype(bf)
        bqk = np.concatenate([b[cs : cs + 256], b[D + cs : D + cs + 256]])
        bqk = np.ascontiguousarray(bqk.reshape(4, 128).T)
        bv = np.ascontiguousarray(b[2 * D + cs : 2 * D + cs + 256].reshape(1, 256))
        in_maps.append(
            {
                "xT": xT[bi],
                "w": np.ascontiguousarray(w_core),
                "bqk": bqk,
                "bv": bv,
            }
        )
    return in_maps


def kernel(x, W_qkv, b_qkv, trace=False):
    from concourse.bass_utils import run_bass_kernel_spmd

    if "nc" not in _CACHED:
        _CACHED["nc"] = build_bass()
    nc = _CACHED["nc"]

    in_maps = _shard_inputs(x, W_qkv, b_qkv)
    res = run_bass_kernel_spmd(nc, in_maps, list(range(8)), trace=trace)
    _CACHED["last_result"] = res

    out = np.empty((2, T, D), dtype=np.float32)
    for c in range(8):
        bi, hg = c // 4, c % 4
        y_raw = res.results[c]["y"]  # [T, 256] unnormalized
        den = res.results[c]["den"]  # [4, T]
        y = y_raw.reshape(T, NH, HD) / den.T[:, :, None]
        out[bi, :, hg * 256 : (hg + 1) * 256] = y.reshape(T, NH * HD)
    return out


if __name__ == "__main__":
    nc = build_bass()
    print("built ok")


# revision 43
# speedup vs baseline: 1.7869x; 1.0038x over previous
"""Trainium2 Bass kernel for fused QKV-projection + multi-head attention.

Problem: x[2,2048,1024] @ W_qkv[1024,3072] + b -> split q/k/v -> 16 heads of
dim 64 -> softmax(q k^T / 8) v -> [2,2048,1024].

Sharding (8 cores): data-parallel over batch (2) x tensor-parallel over head
groups (4 heads per core).  Each core computes a disjoint output slice
[2048, 256]; no collectives are needed.

Design notes:
- Matmul operands are bf16 (fp32 PSUM accumulation): full-rate PE with
  overlapped weight loads. x is pre-transposed and pre-cast on the host, so
  no on-device transposes are needed for the projection.
- qT/kT live in [head-dim, t] layout packed as head PAIRS on the partition
  axis; the scores matmuls use 64-row array tiling (tile_position (0,0) /
  (64,0)) so both heads of a pair run concurrently on the PE array.
- scoresT [k, q] layout: softmax denominator = ones-column appended to V
  ([E^T V | E^T 1] in one PSUM accumulation).  exp has no max-subtraction:
  scores are bounded (~[-3.3, 3.3]) for this problem's scale.
- The final division by the denominator happens on the host; the kernel
  returns the numerator and denominators.
"""

import sys

sys.path.insert(0, "/opt/trn_rl_repo")

import numpy as np

import concourse.bacc as bacc
import concourse.bass as bass
import concourse.mybir as mybir
import concourse.tile as tile
from concourse.bass import ts
from concourse.masks import make_identity

P = 128
T = 2048
D = 1024
NH = 4          # heads per core
HD = 64         # head dim
TB = T // P     # 16 t-blocks
CB = D // P     # 8 c-blocks
QKV_COLS = 3 * NH * HD  # 768 per core
F32 = mybir.dt.float32
BF16 = mybir.dt.bfloat16

_CACHED = {}


def build_bass(finalize=True):
    nc = bacc.Bacc()

    xT_d = nc.dram_tensor("xT", [D, T], BF16, kind="ExternalInput")
    w_d = nc.dram_tensor("w", [D, QKV_COLS], BF16, kind="ExternalInput")
    bqk_d = nc.dram_tensor("bqk", [P, 4], F32, kind="ExternalInput")
    bv_d = nc.dram_tensor("bv", [1, NH * HD], F32, kind="ExternalInput")
    y_d = nc.dram_tensor("y", [T, NH * HD], F32, kind="ExternalOutput")
    den_d = nc.dram_tensor("den", [NH, T], F32, kind="ExternalOutput")

    with tile.TileContext(nc) as tc:
        with (
            tc.tile_pool(name="persist", bufs=1) as persist,
            tc.tile_pool(name="small", bufs=2) as small,
            tc.tile_pool(name="ystage", bufs=4) as ystage,
            tc.tile_pool(name="epool", bufs=3) as epool,
            tc.tile_pool(name="ps_s", bufs=1, space="PSUM") as ps_s,
            tc.tile_pool(name="ps_y", bufs=1, space="PSUM") as ps_y,
        ):
            ident = persist.tile([P, P], F32)
            make_identity(nc, ident)

            # kT: [p, t] per pair; head 2*pr at partitions 0:64, 2*pr+1 at 64:128
            kT = [persist.tile([P, T], BF16, name=f"kT{i}") for i in range(2)]
            # qT: [p, t] per head, zero-padded: head h's 64 dims live at
            # partitions (h%2)*64..+64, the other 64 partitions stay zero so a
            # full-128 matmul against the kT pair tile selects only head h
            qT = [persist.tile([P, T], BF16, name=f"qT{h}") for h in range(NH)]
            for h in range(NH):
                nc.vector.memset(qT[h][:], 0.0)
            # V' with ones column per head: [t-part, h, 65], one tile per tb
            vv = [
                persist.tile([P, NH, HD + 1], BF16, name=f"vv{tb}")
                for tb in range(TB)
            ]
            for tb in range(TB):
                nc.vector.memset(vv[tb][:, :, HD : HD + 1], 1.0)
            bqk_sb = persist.tile([P, 4], F32)
            bvb = persist.tile([P, NH * HD], F32)
            # unnormalized numerator, transposed layout [d-part, t], per pair
            yT = [persist.tile([P, T], F32, name=f"yT{i}") for i in range(2)]

            nc.sync.dma_start(out=bqk_sb[:], in_=bqk_d[:, :])
            nc.gpsimd.dma_start(
                out=bvb[:], in_=bv_d[0:1, :].to_broadcast((P, NH * HD))
            )

            # W split per column group so the first projections' weights land
            # before the whole W transfer completes
            wct = [
                persist.tile([P, CB, P], BF16, name=f"wct{i}") for i in range(4)
            ]
            wv = persist.tile([P, CB, NH * HD], BF16)
            for i in (0, 2):
                nc.sync.dma_start(
                    out=wct[i][:],
                    in_=w_d[:, ts(i, P)].rearrange("(cb p) col -> p cb col", p=P),
                )
            # one tile + DMA per c-block so matmuls start on the first chunk
            xTs = [persist.tile([P, T], BF16, name=f"xTs{cb}") for cb in range(CB)]
            for cb in range(CB):
                nc.sync.dma_start(out=xTs[cb][:], in_=xT_d[ts(cb, P), :])
            nc.sync.dma_start(
                out=wv[:],
                in_=w_d[:, 2 * NH * HD :].rearrange("(cb p) col -> p cb col", p=P),
            )
            for i in (1, 3):
                nc.sync.dma_start(
                    out=wct[i][:],
                    in_=w_d[:, ts(i, P)].rearrange("(cb p) col -> p cb col", p=P),
                )

            # ---------------- QKV projection --------------------------------
            def qk_proj(ct):
                for tc2 in range(4):  # 512-wide t-chunks
                    pqk = ps_s.tile([P, 512], F32, tag=f"S{tc2 % 2}", name="pqk")
                    for cb in range(CB):
                        nc.tensor.matmul(
                            pqk[:],
                            lhsT=wct[ct][:, cb, :],
                            rhs=xTs[cb][:, ts(tc2, 512)],
                            start=(cb == 0),
                            stop=(cb == CB - 1),
                        )
                    if ct < 2:
                        for s in range(2):
                            nc.vector.tensor_scalar_add(
                                out=qT[2 * ct + s][
                                    s * 64 : (s + 1) * 64, ts(tc2, 512)
                                ],
                                in0=pqk[s * 64 : (s + 1) * 64, :],
                                scalar1=bqk_sb[s * 64 : (s + 1) * 64, ct : ct + 1],
                            )
                    else:
                        nc.vector.tensor_scalar_add(
                            out=kT[ct - 2][:, ts(tc2, 512)],
                            in0=pqk[:],
                            scalar1=bqk_sb[:, ct : ct + 1],
                        )

            # pair 0's q/k first so its attention can begin ASAP
            qk_proj(0)
            qk_proj(2)
            for tb in range(TB):
                pv = ps_y.tile(
                    [P, NH * HD], F32, tag=["Y0", "Y1"][tb % 2], name="pv"
                )
                for cb in range(CB):
                    nc.tensor.matmul(
                        pv[:],
                        lhsT=xTs[cb][:, ts(tb, P)],
                        rhs=wv[:, cb, :],
                        start=(cb == 0),
                        stop=(cb == CB - 1),
                    )
                nc.vector.tensor_tensor(
                    out=vv[tb][:, :, 0:HD],
                    in0=pv[:].rearrange("p (a b) -> p a b", a=NH),
                    in1=bvb[:].rearrange("p (a b) -> p a b", a=NH),
                    op=mybir.AluOpType.add,
                )
            # ---------------- attention -------------------------------------
            # Software pipeline per kb: scores(kb) -> exp(kb) on ACT while the
            # PE runs AV(kb-1).  AV is issued AFTER the next scores so the
            # in-order PE queue never stalls behind the exp it feeds.
            def attention(pr):
                for qh in range(2):  # 1024-wide q halves
                    pY = [
                        ps_y.tile([HD + 1, 1024], F32, tag=f"Y{s}", name=f"pY{s}")
                        for s in range(2)
                    ]

                    def issue_av(kb, eprev):
                        for s in range(2):
                            for i in range(2):
                                nc.tensor.matmul(
                                    pY[s][:, ts(i, 512)],
                                    lhsT=vv[kb][:, 2 * pr + s, :],
                                    rhs=eprev[s][:, ts(i, 512)],
                                    start=(kb == 0),
                                    stop=(kb == TB - 1),
                                )

                    prev = None
                    for kb in range(TB):
                        pS = [
                            ps_s.tile([P, 1024], F32, tag=f"S{s}", name=f"pS{s}")
                            for s in range(2)
                        ]
                        for s in range(2):
                            for i in range(2):
                                nc.tensor.matmul(
                                    pS[s][:, ts(i, 512)],
                                    lhsT=kT[pr][:, ts(kb, P)],
                                    rhs=qT[2 * pr + s][
                                        :,
                                        qh * 1024 + i * 512 : qh * 1024 + (i + 1) * 512,
                                    ],
                                    start=True,
                                    stop=True,
                                )
                        eT = [
                            epool.tile([P, 1024], BF16, tag=f"E{s}", name=f"eT{s}")
                            for s in range(2)
                        ]
                        for s in range(2):
                            nc.scalar.activation(
                                out=eT[s][:],
                                in_=pS[s][:],
                                func=mybir.ActivationFunctionType.Exp,
                                scale=0.125,
                            )
                        if prev is not None:
                            issue_av(kb - 1, prev)
                        prev = eT
                    issue_av(TB - 1, prev)
                    for s in range(2):
                        nc.vector.tensor_copy(
                            out=yT[pr][s * 64 : (s + 1) * 64, ts(qh, 1024)],
                            in_=pY[s][0:HD, :],
                        )
                        dsb = small.tile([1, 1024], F32)
                        nc.vector.tensor_copy(out=dsb[:], in_=pY[s][HD : HD + 1, :])
                        nc.sync.dma_start(
                            out=den_d[2 * pr + s : 2 * pr + s + 1, ts(qh, 1024)],
                            in_=dsb[:],
                        )

                    # transpose the just-finished q-half back to [t, d] and
                    # store; borrows the Y psum slots so the S slots stay free
                    for g in range(2):
                        g4 = qh * 2 + g
                        pT = ps_y.tile(
                            [P, 512], F32, tag=["Y0", "Y1"][g % 2], name="pT"
                        )
                        for j in range(4):
                            nc.tensor.transpose(
                                pT[:, ts(j, P)], yT[pr][:, ts(4 * g4 + j, P)], ident
                            )
                        yst = ystage.tile([P, 4, P], F32, name="yst")
                        nc.vector.tensor_copy(
                            out=yst[:], in_=pT[:].rearrange("p (a b) -> p a b", a=4)
                        )
                        for j in range(4):
                            nc.sync.dma_start(
                                out=y_d[ts(4 * g4 + j, P), ts(pr, P)],
                                in_=yst[:, j, :],
                            )

            attention(0)
            qk_proj(1)
            qk_proj(3)
            attention(1)

    if finalize:
        nc.finalize()
    return nc


def _shard_inputs(x, W_qkv, b_qkv):
    """Build per-core input maps. Core c: batch c//4, head group c%4."""
    import ml_dtypes

    x = np.asarray(x, dtype=np.float32)
    W = np.asarray(W_qkv, dtype=np.float32)
    b = np.asarray(b_qkv, dtype=np.float32)
    bf = ml_dtypes.bfloat16
    xT = [np.ascontiguousarray(x[bi].T.astype(bf)) for bi in range(2)]
    in_maps = []
    for c in range(8):
        bi, hg = c // 4, c % 4
        cs = hg * 256  # column start within each of q/k/v blocks
        w_core = np.concatenate(
            [
                W[:, cs : cs + 256],
                W[:, D + cs : D + cs + 256],
                W[:, 2 * D + cs : 2 * D + cs + 256],
            ],
            axis=1,
        ).astype(bf)
        bqk = np.concatenate([b[cs : cs + 256], b[D + cs : D + cs + 256]])
        bqk = np.ascontiguousarray(bqk.reshape(4, 128).T)
        bv = np.ascontiguousarray(b[2 * D + cs : 2 * D + cs + 256].reshape(1, 256))
        in_maps.append(
            {
                "xT": xT[bi],
                "w": np.ascontiguousarray(w_core),
                "bqk": bqk,
                "bv": bv,
            }
        )
    return in_maps


def kernel(x, W_qkv, b_qkv, trace=False):
    from concourse.bass_utils import run_bass_kernel_spmd

    if "nc" not in _CACHED:
        _CACHED["nc"] = build_bass()
    nc = _CACHED["nc"]

    in_maps = _shard_inputs(x, W_qkv, b_qkv)
    res = run_bass_kernel_spmd(nc, in_maps, list(range(8)), trace=trace)
    _CACHED["last_result"] = res

    out = np.empty((2, T, D), dtype=np.float32)
    for c in range(8):
        bi, hg = c // 4, c % 4
        y_raw = res.results[c]["y"]  # [T, 256] unnormalized
        den = res.results[c]["den"]  # [4, T]
        y = y_raw.reshape(T, NH, HD) / den.T[:, :, None]
        out[bi, :, hg * 256 : (hg + 1) * 256] = y.reshape(T, NH * HD)
    return out


if __name__ == "__main__":
    nc = build_bass()
    print("built ok")
